# revision 33
# baseline (speedup 1.0000x reference)
"""Distributed Bass kernel for cluster sparse-attention on 8 TRN2 NeuronCores.

Reference math (b=2, n=2048, nc=256, d=512, 16 heads x 32):
  q = heads(concat([x, clusters]) @ Wq); k,v = heads(x @ Wkv)
  S = q @ k^T * scale
  token queries:   softmax over keys (the log-token_sizes bias is constant
                   per softmax row, so it cancels exactly) -> @ v -> @ Wo
  cluster queries: softmax over the CLUSTER axis, then row-normalized by
                   (sum over keys + 1e-5) -> @ v -> @ Wo

Sharding: core c -> (batch b = c//4, head-group g = c%4, i.e. heads 4g..4g+4).
Each core computes its 4 heads' attention output transposed (AOT, [128, 2304])
and multiplies by its 128 rows of Wo, giving a partial [2304, 512]; the four
partials of a batch are summed on the host (pure unshard of the sum-sharded
output), then split into (x_out, clusters_out).

Device layout trick: scores are computed transposed, S^T[keys, queries], so
softmax needs no PE transposes: exp runs without max-subtraction (scores are
O(1) here; verified offline), denominators come from an all-ones lhsT matmul
(column sums across the key/partition axis), and the cluster softmax is a
free-axis normalize. AV then contracts keys on the partition axis directly.
"""

import numpy as np

import concourse.bacc as bacc
import concourse.mybir as mybir
from concourse.tile import TileContext
from concourse.bass_utils import run_bass_kernel_spmd

B, N, NCL, D = 2, 2048, 256, 512
NQ = N + NCL              # 2304 packed queries (tokens then clusters)
SCALE = 32.0 ** -0.5
EPS_C = 1e-5
FP32 = mybir.dt.float32
BF16 = mybir.dt.bfloat16
AF = mybir.ActivationFunctionType

# (query offset, width) pieces; last piece is the cluster queries.
# 1024-wide pieces amortize the ~293ns per-ACTIVATE overhead over N=1024 exps.
QPIECES = [(0, 1024), (1024, 1024), (2048, 256)]


def build_graph(debug=False):
    nc = bacc.Bacc()
    xT = nc.declare_dram_parameter("xT", [D, N], FP32, isOutput=False)
    clT = nc.declare_dram_parameter("clT", [D, NCL], FP32, isOutput=False)
    wq = nc.declare_dram_parameter("wq", [128, 512], FP32, isOutput=False)
    wk = nc.declare_dram_parameter("wk", [128, 512], FP32, isOutput=False)
    wv = nc.declare_dram_parameter("wv", [128, 512], FP32, isOutput=False)
    wo = nc.declare_dram_parameter("wo", [128, 512], FP32, isOutput=False)
    bcm = nc.declare_dram_parameter("bcm", [4, 128], FP32, isOutput=False)
    out = nc.declare_dram_parameter("out", [NQ, D], FP32, isOutput=True)

    with TileContext(nc) as tc:
        with tc.sbuf_pool(name="sb1", bufs=1) as sb1, \
             tc.sbuf_pool(name="work", bufs=3) as work, \
             tc.psum_pool(name="ps", bufs=2) as psp, \
             tc.psum_pool(name="avp", bufs=1) as avp:

            # ---------------- input DMA ----------------
            # Everything the PE reads is staged through one DVE copy per DMA:
            # matmul (fp32 self-loading LDW) only supports a single sync-wait,
            # so all its input tiles must be produced by a single engine
            # (cumulative thresholds on the one DVE semaphore collapse).
            xT_st = sb1.tile([128, 4 * N], FP32)
            xT_sb = sb1.tile([128, 4 * N], BF16)     # d-chunk c at cols [c*N, (c+1)*N)
            for c in range(4):
                nc.sync.dma_start(xT_st[:, c * N:(c + 1) * N], xT[128 * c:128 * (c + 1), :])
                nc.vector.tensor_copy(xT_sb[:, c * N:(c + 1) * N], xT_st[:, c * N:(c + 1) * N])
            clT_st = sb1.tile([128, 4 * NCL], FP32)
            clT_sb = sb1.tile([128, 4 * NCL], BF16)
            for c in range(4):
                nc.sync.dma_start(clT_st[:, c * NCL:(c + 1) * NCL], clT[128 * c:128 * (c + 1), :])
                nc.vector.tensor_copy(clT_sb[:, c * NCL:(c + 1) * NCL], clT_st[:, c * NCL:(c + 1) * NCL])

            def load_weight(param, name, dtype=FP32):
                st = sb1.tile([128, 512], FP32, name=name + "_st")
                fin = sb1.tile([128, 512], dtype, name=name + "_sb")
                nc.sync.dma_start(st[:, :], param[:, :])
                nc.vector.tensor_copy(fin[:, :], st[:, :])
                return fin

            wq_sb = load_weight(wq, "wq", BF16)
            wk_sb = load_weight(wk, "wk", BF16)
            wv_sb = load_weight(wv, "wv", BF16)
            wo_sb = load_weight(wo, "wo", BF16)
            bcm_st = sb1.tile([4, 128], FP32)
            nc.sync.dma_start(bcm_st[:, :], bcm[:, :])
            bcm_sb = sb1.tile([4, 128], BF16)
            nc.vector.tensor_copy(bcm_sb[:, :], bcm_st[:, :])

            # ---------------- projections ----------------
            # kT/qT: [head-dim (4h x 32), seq] with head h at partitions 32h.
            kT_sb = sb1.tile([128, N], BF16)
            for ns in range(4):
                kps = psp.tile([128, 512], FP32, tag="sc", name="kps")
                for c in range(4):
                    nc.tensor.matmul(kps[:, :], wk_sb[:, 128 * c:128 * (c + 1)],
                                     xT_sb[:, c * N + 512 * ns: c * N + 512 * (ns + 1)],
                                     start=(c == 0), stop=(c == 3))
                nc.vector.tensor_copy(kT_sb[:, 512 * ns:512 * (ns + 1)], kps[:, :])

            qT_sb = sb1.tile([128, NQ], BF16)
            for ns in range(4):
                qps = psp.tile([128, 512], FP32, tag="sc", name="qps")
                for c in range(4):
                    nc.tensor.matmul(qps[:, :], wq_sb[:, 128 * c:128 * (c + 1)],
                                     xT_sb[:, c * N + 512 * ns: c * N + 512 * (ns + 1)],
                                     start=(c == 0), stop=(c == 3))
                nc.vector.tensor_copy(qT_sb[:, 512 * ns:512 * (ns + 1)], qps[:, :])
            cps = psp.tile([128, NCL], FP32, tag="sc", name="cps")
            for c in range(4):
                nc.tensor.matmul(cps[:, :], wq_sb[:, 128 * c:128 * (c + 1)],
                                 clT_sb[:, c * NCL:(c + 1) * NCL],
                                 start=(c == 0), stop=(c == 3))
            nc.vector.tensor_copy(qT_sb[:, N:NQ], cps[:, :])

            # v in natural [keys, head-dim] layout, 33 cols/head: col 33h+32 is
            # the all-ones column whose AV output row is the softmax denominator.
            v_sb = sb1.tile([128, 16 * 132], BF16)
            nc.vector.memset(v_sb[:, :], 1.0)
            for kc in range(16):
                vps = psp.tile([128, 128], FP32, tag="sc", name="vps")
                for c in range(4):
                    nc.tensor.matmul(vps[:, :],
                                     xT_sb[:, c * N + 128 * kc: c * N + 128 * (kc + 1)],
                                     wv_sb[:, 128 * c:128 * (c + 1)],
                                     start=(c == 0), stop=(c == 3))
                dst = v_sb[:, 132 * kc:132 * kc + 132].rearrange("p (h x) -> p h x", h=4)[:, :, 0:32]
                nc.vector.tensor_copy(dst, vps.rearrange("p (h x) -> p h x", h=4))

            # ---------------- attention ----------------
            AOT_sb = sb1.tile([128, NQ], BF16)   # normalized attn output^T, head h at 32h

            def normalize_and_wo(qoff, W, is_cl, avA, avB):
                # normalize: AOT[32h+i] = av_out[h][i] / den[h]  (+1e-5 for
                # clusters).  DVE ops must start at partition 0 on this stack,
                # so: copy av to SBUF, DMA-assemble the den rows into [4, W]
                # and the out rows into head-order, reciprocal, broadcast the
                # reciprocal rows 4->128 with a K=4 matmul against the 0/1
                # bcm matrix, then one full-width multiply.
                nsl = [(s, min(512, W - s)) for s in range(0, W, 512)]
                avsbA = work.tile([128, W], FP32, tag="avsbA", name="avsbA", bufs=2)
                avsbB = work.tile([128, W], FP32, tag="avsbB", name="avsbB", bufs=2)
                nc.vector.tensor_copy(avsbA[:, :], avA[:, :])
                nc.vector.tensor_copy(avsbB[:, :], avB[:, :])
                den4 = work.tile([4, W], FP32, tag="den4", name="den4", bufs=2)
                for h, (src, row) in enumerate(((avsbA, 32), (avsbA, 96),
                                                (avsbB, 32), (avsbB, 96))):
                    nc.sync.dma_start(den4[h:h + 1, :], src[row:row + 1, :])
                if is_cl:
                    nc.vector.tensor_scalar_add(den4[:, :], den4[:, :], EPS_C)
                rden4 = work.tile([4, W], FP32, tag="rden4", name="rden4", bufs=2)
                nc.vector.reciprocal_approx_fast(rden4[:, :], den4[:, :])
                rden4b = work.tile([4, W], BF16, tag="rden4b", name="rden4b", bufs=2)
                nc.vector.tensor_copy(rden4b[:, :], rden4[:, :])
                AOTraw = work.tile([128, W], FP32, tag="AOTraw", name="AOTraw", bufs=2)
                for h, (src, row) in enumerate(((avsbA, 0), (avsbA, 64),
                                                (avsbB, 0), (avsbB, 64))):
                    nc.sync.dma_start(AOTraw[32 * h:32 * (h + 1), :], src[row:row + 32, :])
                bcsb = work.tile([128, W], FP32, tag="bcsb", name="bcsb", bufs=2)
                for (so, sw) in nsl:
                    bcp = psp.tile([128, 512], FP32, tag="sc", name="bcp")
                    nc.tensor.matmul(bcp[:, :sw], bcm_sb[:, :], rden4b[:, so:so + sw],
                                     start=True, stop=True)
                    nc.vector.tensor_copy(bcsb[:, so:so + sw], bcp[:, :sw])
                nc.vector.tensor_mul(AOT_sb[:, qoff:qoff + W], AOTraw[:, :], bcsb[:, :])
                # Wo for this piece's output rows + partial-output DMA
                for j in range(W // 128):
                    row0 = qoff + 128 * j
                    ops = psp.tile([128, 512], FP32, tag="sc", name="ops")
                    nc.tensor.matmul(ops[:, :], AOT_sb[:, row0:row0 + 128], wo_sb[:, :],
                                     start=True, stop=True)
                    osb = work.tile([128, 512], FP32, tag="osb", name="osb")
                    nc.vector.tensor_copy(osb[:, :], ops[:, :])
                    nc.sync.dma_start(out[row0:row0 + 128, :], osb[:, :])

            # Each piece's normalize+Wo is EMITTED in the middle of the next
            # piece's kc loop: the engine queues are in-order, so emitting it
            # at the piece boundary would head-block the PE FIFO on the serial
            # DVE normalize chain (~5us) — long enough for HAM to re-throttle
            # the PE clock to 1.2 GHz for the rest of the kernel.
            pending = None
            for (qoff, W) in QPIECES:
                is_cl = (qoff == N)
                nsl = [(s, min(512, W - s)) for s in range(0, W, 512)]
                # avA: h0 rows 0..32 (+den row 32), h1 rows 64..96 (+den 96);
                # avB: h2, h3 likewise.  M=33 AV includes the ones-column den.
                avA = avp.tile([128, W], FP32, tag="av", name="avA")
                avB = avp.tile([128, W], FP32, tag="den", name="avB")
                for kc in range(16):
                    if kc == 5 and pending is not None:
                        pending()
                        pending = None
                    for h in range(4):
                        sps = psp.tile([128, W], FP32, tag="sc", name="sps")
                        for (so, sw) in nsl:
                            nc.tensor.matmul(sps[:, so:so + sw],
                                             kT_sb[32 * h:32 * (h + 1), 128 * kc:128 * (kc + 1)],
                                             qT_sb[32 * h:32 * (h + 1), qoff + so:qoff + so + sw],
                                             start=True, stop=True,
                                             tile_position=(32 * h, 0))
                        PT = work.tile([128, W], BF16, tag="PT", name="PT")
                        nc.scalar.activation(PT[:, :], sps[:, :], AF.Exp, scale=SCALE)
                        if is_cl:
                            # softmax over the cluster (free) axis per key row
                            rsum = work.tile([128, 1], FP32, tag="rsum", name="rsum")
                            nc.vector.reduce_sum(out=rsum[:, :], in_=PT[:, :],
                                                 axis=mybir.AxisListType.X)
                            rinv = work.tile([128, 1], FP32, tag="rinv", name="rinv")
                            nc.vector.reciprocal(rinv[:, :], rsum[:, :])
                            nc.vector.tensor_scalar_mul(PT[:, :], PT[:, :], rinv[:, :])
                        avX = avA if h < 2 else avB
                        off = 64 * (h % 2)
                        for (so, sw) in nsl:
                            nc.tensor.matmul(avX[off:off + 33, so:so + sw],
                                             v_sb[:, 132 * kc + 33 * h:132 * kc + 33 * h + 33],
                                             PT[:, so:so + sw],
                                             start=(kc == 0), stop=(kc == 15),
                                             tile_position=(0, off))
                pending = (lambda q=qoff, w=W, c=is_cl, a=avA, b=avB:
                           normalize_and_wo(q, w, c, a, b))
            pending()

            if debug:
                dbg_kT = nc.declare_dram_parameter("dbg_kT", [128, N], FP32, isOutput=True)
                nc.sync.dma_start(dbg_kT[:, :], kT_sb[:, :])
                dbg_qT = nc.declare_dram_parameter("dbg_qT", [128, NQ], FP32, isOutput=True)
                nc.sync.dma_start(dbg_qT[:, :], qT_sb[:, :])
                dbg_v = nc.declare_dram_parameter("dbg_v", [128, 2048], FP32, isOutput=True)
                nc.sync.dma_start(dbg_v[:, :], v_sb[:, :])
                dbg_AOT = nc.declare_dram_parameter("dbg_AOT", [128, NQ], FP32, isOutput=True)
                nc.sync.dma_start(dbg_AOT[:, :], AOT_sb[:, :])
    nc.compile()
    return nc


def _shard_weights(Wq, Wkv, Wo, g):
    gs = slice(128 * g, 128 * (g + 1))

    def chunked(w):  # [512, 128] -> [128, 512] with d-chunk c at cols 128c
        return np.ascontiguousarray(
            w.reshape(4, 128, 128).transpose(1, 0, 2).reshape(128, 512))

    wq_c = chunked(np.ascontiguousarray(Wq[:, gs]))
    wk_c = chunked(np.ascontiguousarray(Wkv[:, :D][:, gs]))
    wv_c = chunked(np.ascontiguousarray(Wkv[:, D:][:, gs]))
    wo_c = np.ascontiguousarray(Wo[gs, :])
    return wq_c, wk_c, wv_c, wo_c


def _run(inputs, trace=False, debug=False):
    x = np.asarray(inputs["x"], np.float32)
    clusters = np.asarray(inputs["clusters"], np.float32)
    Wq = np.asarray(inputs["Wq"], np.float32)
    Wkv = np.asarray(inputs["Wkv"], np.float32)
    Wo = np.asarray(inputs["Wo"], np.float32)

    nc = build_graph(debug=debug)
    bcm = np.zeros((4, 128), np.float32)
    for h in range(4):
        bcm[h, 32 * h:32 * (h + 1)] = 1.0
    in_maps = []
    for core in range(8):
        b, g = core // 4, core % 4
        wq_c, wk_c, wv_c, wo_c = _shard_weights(Wq, Wkv, Wo, g)
        in_maps.append({
            "xT": np.ascontiguousarray(x[b].T),
            "clT": np.ascontiguousarray(clusters[b].T),
            "wq": wq_c, "wk": wk_c, "wv": wv_c, "wo": wo_c, "bcm": bcm,
        })
    bkr = run_bass_kernel_spmd(nc, in_maps, core_ids=list(range(8)), trace=trace)
    parts = [bkr.results[i]["out"] for i in range(8)]
    x_out = np.stack([sum(parts[4 * b:4 * b + 4])[:N] for b in range(B)])
    cl_out = np.stack([sum(parts[4 * b:4 * b + 4])[N:] for b in range(B)])
    return (x_out, cl_out), bkr


def kernel(x, clusters, token_sizes, Wq, Wkv, Wo):
    outs, _ = _run(dict(x=x, clusters=clusters, Wq=Wq, Wkv=Wkv, Wo=Wo))
    return outs


# revision 34
# speedup vs baseline: 1.2199x; 1.2199x over previous
"""Distributed Bass kernel for cluster sparse-attention on 8 TRN2 NeuronCores.

Reference math (b=2, n=2048, nc=256, d=512, 16 heads x 32):
  q = heads(concat([x, clusters]) @ Wq); k,v = heads(x @ Wkv)
  S = q @ k^T * scale
  token queries:   softmax over keys (the log-token_sizes bias is constant
                   per softmax row, so it cancels exactly) -> @ v -> @ Wo
  cluster queries: softmax over the CLUSTER axis, then row-normalized by
                   (sum over keys + 1e-5) -> @ v -> @ Wo

Sharding: core c -> (batch b = c//4, head-group g = c%4, i.e. heads 4g..4g+4).
Each core computes its 4 heads' attention output transposed (AOT, [128, 2304])
and multiplies by its 128 rows of Wo, giving a partial [2304, 512]; the four
partials of a batch are summed on the host (pure unshard of the sum-sharded
output), then split into (x_out, clusters_out).

Device layout trick: scores are computed transposed, S^T[keys, queries], so
softmax needs no PE transposes: exp runs without max-subtraction (scores are
O(1) here; verified offline), denominators come from an all-ones lhsT matmul
(column sums across the key/partition axis), and the cluster softmax is a
free-axis normalize. AV then contracts keys on the partition axis directly.
"""

import numpy as np

import concourse.bacc as bacc
import concourse.mybir as mybir
from concourse.tile import TileContext
from concourse.bass_utils import run_bass_kernel_spmd

B, N, NCL, D = 2, 2048, 256, 512
NQ = N + NCL              # 2304 packed queries (tokens then clusters)
SCALE = 32.0 ** -0.5
EPS_C = 1e-5
FP32 = mybir.dt.float32
BF16 = mybir.dt.bfloat16
AF = mybir.ActivationFunctionType

# (query offset, width) pieces; last piece is the cluster queries.
# 1024-wide pieces amortize the ~293ns per-ACTIVATE overhead over N=1024 exps.
QPIECES = [(0, 1024), (1024, 1024), (2048, 256)]


def build_graph(debug=False):
    nc = bacc.Bacc()
    xT = nc.declare_dram_parameter("xT", [D, N], FP32, isOutput=False)
    clT = nc.declare_dram_parameter("clT", [D, NCL], FP32, isOutput=False)
    wq = nc.declare_dram_parameter("wq", [128, 512], FP32, isOutput=False)
    wk = nc.declare_dram_parameter("wk", [128, 512], FP32, isOutput=False)
    wv = nc.declare_dram_parameter("wv", [128, 512], FP32, isOutput=False)
    wo = nc.declare_dram_parameter("wo", [128, 512], FP32, isOutput=False)
    bcm = nc.declare_dram_parameter("bcm", [4, 128], FP32, isOutput=False)
    out = nc.declare_dram_parameter("out", [NQ, D], FP32, isOutput=True)

    with TileContext(nc) as tc:
        with tc.sbuf_pool(name="sb1", bufs=1) as sb1, \
             tc.sbuf_pool(name="work", bufs=3) as work, \
             tc.psum_pool(name="ps", bufs=2) as psp, \
             tc.psum_pool(name="avp", bufs=1) as avp:

            # ---------------- input DMA ----------------
            # Everything the PE reads is staged through one DVE copy per DMA:
            # matmul (fp32 self-loading LDW) only supports a single sync-wait,
            # so all its input tiles must be produced by a single engine
            # (cumulative thresholds on the one DVE semaphore collapse).
            xT_st = sb1.tile([128, 4 * N], FP32)
            xT_sb = sb1.tile([128, 4 * N], BF16)     # d-chunk c at cols [c*N, (c+1)*N)
            for c in range(4):
                nc.sync.dma_start(xT_st[:, c * N:(c + 1) * N], xT[128 * c:128 * (c + 1), :])
                nc.vector.tensor_copy(xT_sb[:, c * N:(c + 1) * N], xT_st[:, c * N:(c + 1) * N])
            clT_st = sb1.tile([128, 4 * NCL], FP32)
            clT_sb = sb1.tile([128, 4 * NCL], BF16)
            for c in range(4):
                nc.sync.dma_start(clT_st[:, c * NCL:(c + 1) * NCL], clT[128 * c:128 * (c + 1), :])
                nc.vector.tensor_copy(clT_sb[:, c * NCL:(c + 1) * NCL], clT_st[:, c * NCL:(c + 1) * NCL])

            def load_weight(param, name, dtype=FP32):
                st = sb1.tile([128, 512], FP32, name=name + "_st")
                fin = sb1.tile([128, 512], dtype, name=name + "_sb")
                nc.sync.dma_start(st[:, :], param[:, :])
                nc.vector.tensor_copy(fin[:, :], st[:, :])
                return fin

            wq_sb = load_weight(wq, "wq", BF16)
            wk_sb = load_weight(wk, "wk", BF16)
            wv_sb = load_weight(wv, "wv", BF16)
            wo_sb = load_weight(wo, "wo", BF16)
            bcm_st = sb1.tile([4, 128], FP32)
            nc.sync.dma_start(bcm_st[:, :], bcm[:, :])
            bcm_sb = sb1.tile([4, 128], BF16)
            nc.vector.tensor_copy(bcm_sb[:, :], bcm_st[:, :])

            # ---------------- projections ----------------
            # kT/qT: [head-dim (4h x 32), seq] with head h at partitions 32h.
            kT_sb = sb1.tile([128, N], BF16)
            for ns in range(4):
                kps = psp.tile([128, 512], FP32, tag="sc", name="kps")
                for c in range(4):
                    nc.tensor.matmul(kps[:, :], wk_sb[:, 128 * c:128 * (c + 1)],
                                     xT_sb[:, c * N + 512 * ns: c * N + 512 * (ns + 1)],
                                     start=(c == 0), stop=(c == 3))
                nc.vector.tensor_copy(kT_sb[:, 512 * ns:512 * (ns + 1)], kps[:, :])

            qT_sb = sb1.tile([128, NQ], BF16)
            for ns in range(4):
                qps = psp.tile([128, 512], FP32, tag="sc", name="qps")
                for c in range(4):
                    nc.tensor.matmul(qps[:, :], wq_sb[:, 128 * c:128 * (c + 1)],
                                     xT_sb[:, c * N + 512 * ns: c * N + 512 * (ns + 1)],
                                     start=(c == 0), stop=(c == 3))
                nc.vector.tensor_copy(qT_sb[:, 512 * ns:512 * (ns + 1)], qps[:, :])
            cps = psp.tile([128, NCL], FP32, tag="sc", name="cps")
            for c in range(4):
                nc.tensor.matmul(cps[:, :], wq_sb[:, 128 * c:128 * (c + 1)],
                                 clT_sb[:, c * NCL:(c + 1) * NCL],
                                 start=(c == 0), stop=(c == 3))
            nc.vector.tensor_copy(qT_sb[:, N:NQ], cps[:, :])

            # v in natural [keys, head-dim] layout, 33 cols/head: col 33h+32 is
            # the all-ones column whose AV output row is the softmax denominator.
            v_sb = sb1.tile([128, 16 * 132], BF16)
            nc.vector.memset(v_sb[:, :], 1.0)
            for kc in range(16):
                vps = psp.tile([128, 128], FP32, tag="sc", name="vps")
                for c in range(4):
                    nc.tensor.matmul(vps[:, :],
                                     xT_sb[:, c * N + 128 * kc: c * N + 128 * (kc + 1)],
                                     wv_sb[:, 128 * c:128 * (c + 1)],
                                     start=(c == 0), stop=(c == 3))
                dst = v_sb[:, 132 * kc:132 * kc + 132].rearrange("p (h x) -> p h x", h=4)[:, :, 0:32]
                nc.vector.tensor_copy(dst, vps.rearrange("p (h x) -> p h x", h=4))

            # ---------------- attention ----------------
            AOT_sb = sb1.tile([128, NQ], BF16)   # normalized attn output^T, head h at 32h

            def normalize_and_wo(qoff, W, is_cl, avA, avB):
                # normalize: AOT[32h+i] = av_out[h][i] / den[h]  (+1e-5 for
                # clusters).  DVE ops must start at partition 0 on this stack,
                # so: copy av to SBUF, DMA-assemble the den rows into [4, W]
                # and the out rows into head-order, reciprocal, broadcast the
                # reciprocal rows 4->128 with a K=4 matmul against the 0/1
                # bcm matrix, then one full-width multiply.
                nsl = [(s, min(512, W - s)) for s in range(0, W, 512)]
                avsbA = work.tile([128, W], FP32, tag="avsbA", name="avsbA", bufs=2)
                avsbB = work.tile([128, W], FP32, tag="avsbB", name="avsbB", bufs=2)
                nc.vector.tensor_copy(avsbA[:, :], avA[:, :])
                nc.vector.tensor_copy(avsbB[:, :], avB[:, :])
                den4 = work.tile([4, W], FP32, tag="den4", name="den4", bufs=2)
                for h, (src, row) in enumerate(((avsbA, 32), (avsbA, 96),
                                                (avsbB, 32), (avsbB, 96))):
                    nc.sync.dma_start(den4[h:h + 1, :], src[row:row + 1, :])
                if is_cl:
                    nc.vector.tensor_scalar_add(den4[:, :], den4[:, :], EPS_C)
                rden4 = work.tile([4, W], FP32, tag="rden4", name="rden4", bufs=2)
                nc.vector.reciprocal_approx_fast(rden4[:, :], den4[:, :])
                rden4b = work.tile([4, W], BF16, tag="rden4b", name="rden4b", bufs=2)
                nc.vector.tensor_copy(rden4b[:, :], rden4[:, :])
                AOTraw = work.tile([128, W], FP32, tag="AOTraw", name="AOTraw", bufs=2)
                for h, (src, row) in enumerate(((avsbA, 0), (avsbA, 64),
                                                (avsbB, 0), (avsbB, 64))):
                    nc.sync.dma_start(AOTraw[32 * h:32 * (h + 1), :], src[row:row + 32, :])
                bcsb = work.tile([128, W], FP32, tag="bcsb", name="bcsb", bufs=2)
                for (so, sw) in nsl:
                    bcp = psp.tile([128, 512], FP32, tag="sc", name="bcp")
                    nc.tensor.matmul(bcp[:, :sw], bcm_sb[:, :], rden4b[:, so:so + sw],
                                     start=True, stop=True)
                    nc.vector.tensor_copy(bcsb[:, so:so + sw], bcp[:, :sw])
                nc.vector.tensor_mul(AOT_sb[:, qoff:qoff + W], AOTraw[:, :], bcsb[:, :])
                # Wo for this piece's output rows + partial-output DMA
                for j in range(W // 128):
                    row0 = qoff + 128 * j
                    ops = psp.tile([128, 512], FP32, tag="sc", name="ops")
                    nc.tensor.matmul(ops[:, :], AOT_sb[:, row0:row0 + 128], wo_sb[:, :],
                                     start=True, stop=True)
                    osb = work.tile([128, 512], FP32, tag="osb", name="osb")
                    nc.vector.tensor_copy(osb[:, :], ops[:, :])
                    nc.sync.dma_start(out[row0:row0 + 128, :], osb[:, :])

            # Each piece's normalize+Wo is EMITTED in the middle of the next
            # piece's kc loop: the engine queues are in-order, so emitting it
            # at the piece boundary would head-block the PE FIFO on the serial
            # DVE normalize chain (~5us) — long enough for HAM to re-throttle
            # the PE clock to 1.2 GHz for the rest of the kernel.
            pending = None
            for (qoff, W) in QPIECES:
                is_cl = (qoff == N)
                nsl = [(s, min(512, W - s)) for s in range(0, W, 512)]
                # avA: h0 rows 0..32 (+den row 32), h1 rows 64..96 (+den 96);
                # avB: h2, h3 likewise.  M=33 AV includes the ones-column den.
                avA = avp.tile([128, W], FP32, tag="av", name="avA")
                avB = avp.tile([128, W], FP32, tag="den", name="avB")
                for kc in range(16):
                    if kc == 3 and pending is not None:
                        pending()
                        pending = None
                    for h in range(4):
                        sps = psp.tile([128, W], FP32, tag="sc", name="sps")
                        for (so, sw) in nsl:
                            nc.tensor.matmul(sps[:, so:so + sw],
                                             kT_sb[32 * h:32 * (h + 1), 128 * kc:128 * (kc + 1)],
                                             qT_sb[32 * h:32 * (h + 1), qoff + so:qoff + so + sw],
                                             start=True, stop=True,
                                             tile_position=(32 * h, 0))
                        PT = work.tile([128, W], BF16, tag="PT", name="PT", bufs=4)
                        nc.scalar.activation(PT[:, :], sps[:, :], AF.Exp, scale=SCALE)
                        if is_cl:
                            # softmax over the cluster (free) axis per key row
                            rsum = work.tile([128, 1], FP32, tag="rsum", name="rsum")
                            nc.vector.reduce_sum(out=rsum[:, :], in_=PT[:, :],
                                                 axis=mybir.AxisListType.X)
                            rinv = work.tile([128, 1], FP32, tag="rinv", name="rinv")
                            nc.vector.reciprocal(rinv[:, :], rsum[:, :])
                            nc.vector.tensor_scalar_mul(PT[:, :], PT[:, :], rinv[:, :])
                        avX = avA if h < 2 else avB
                        off = 64 * (h % 2)
                        for (so, sw) in nsl:
                            nc.tensor.matmul(avX[off:off + 33, so:so + sw],
                                             v_sb[:, 132 * kc + 33 * h:132 * kc + 33 * h + 33],
                                             PT[:, so:so + sw],
                                             start=(kc == 0), stop=(kc == 15),
                                             tile_position=(0, off))
                pending = (lambda q=qoff, w=W, c=is_cl, a=avA, b=avB:
                           normalize_and_wo(q, w, c, a, b))
            pending()

            if debug:
                dbg_kT = nc.declare_dram_parameter("dbg_kT", [128, N], FP32, isOutput=True)
                nc.sync.dma_start(dbg_kT[:, :], kT_sb[:, :])
                dbg_qT = nc.declare_dram_parameter("dbg_qT", [128, NQ], FP32, isOutput=True)
                nc.sync.dma_start(dbg_qT[:, :], qT_sb[:, :])
                dbg_v = nc.declare_dram_parameter("dbg_v", [128, 2048], FP32, isOutput=True)
                nc.sync.dma_start(dbg_v[:, :], v_sb[:, :])
                dbg_AOT = nc.declare_dram_parameter("dbg_AOT", [128, NQ], FP32, isOutput=True)
                nc.sync.dma_start(dbg_AOT[:, :], AOT_sb[:, :])
    nc.compile()
    return nc


def _shard_weights(Wq, Wkv, Wo, g):
    gs = slice(128 * g, 128 * (g + 1))

    def chunked(w):  # [512, 128] -> [128, 512] with d-chunk c at cols 128c
        return np.ascontiguousarray(
            w.reshape(4, 128, 128).transpose(1, 0, 2).reshape(128, 512))

    wq_c = chunked(np.ascontiguousarray(Wq[:, gs]))
    wk_c = chunked(np.ascontiguousarray(Wkv[:, :D][:, gs]))
    wv_c = chunked(np.ascontiguousarray(Wkv[:, D:][:, gs]))
    wo_c = np.ascontiguousarray(Wo[gs, :])
    return wq_c, wk_c, wv_c, wo_c


def _run(inputs, trace=False, debug=False):
    x = np.asarray(inputs["x"], np.float32)
    clusters = np.asarray(inputs["clusters"], np.float32)
    Wq = np.asarray(inputs["Wq"], np.float32)
    Wkv = np.asarray(inputs["Wkv"], np.float32)
    Wo = np.asarray(inputs["Wo"], np.float32)

    nc = build_graph(debug=debug)
    bcm = np.zeros((4, 128), np.float32)
    for h in range(4):
        bcm[h, 32 * h:32 * (h + 1)] = 1.0
    in_maps = []
    for core in range(8):
        b, g = core // 4, core % 4
        wq_c, wk_c, wv_c, wo_c = _shard_weights(Wq, Wkv, Wo, g)
        in_maps.append({
            "xT": np.ascontiguousarray(x[b].T),
            "clT": np.ascontiguousarray(clusters[b].T),
            "wq": wq_c, "wk": wk_c, "wv": wv_c, "wo": wo_c, "bcm": bcm,
        })
    bkr = run_bass_kernel_spmd(nc, in_maps, core_ids=list(range(8)), trace=trace)
    parts = [bkr.results[i]["out"] for i in range(8)]
    x_out = np.stack([sum(parts[4 * b:4 * b + 4])[:N] for b in range(B)])
    cl_out = np.stack([sum(parts[4 * b:4 * b + 4])[N:] for b in range(B)])
    return (x_out, cl_out), bkr


def kernel(x, clusters, token_sizes, Wq, Wkv, Wo):
    outs, _ = _run(dict(x=x, clusters=clusters, Wq=Wq, Wkv=Wkv, Wo=Wo))
    return outs
